# revision 14
# baseline (speedup 1.0000x reference)
"""Causal multi-head attention (B=4, N=2048, C=1024, H=16) on 8 Trainium2 cores.

Sharding: data-parallel over batch (4) x tensor-parallel over heads (2 groups
of 8).  Core c handles batch c//2, head-group c%2.  Each core computes its
heads' attention and a partial output projection; the host sums the two
head-group partials per batch and adds the bias.

Device layout notes (per core):
  - All matmul operands are bf16; accumulation fp32 in PSUM.
  - x, weights are shipped pre-transposed so QKV lands as q^T/k^T [d, n].
  - Scores are computed transposed (S^T[kv, q]) so softmax's exp feeds the
    PV matmul directly without transposing the probability matrix.
  - No max-subtraction in softmax: scores are O(1) (std ~1) by construction,
    exp never overflows fp32.  The causal mask is added via an
    identity-matmul of an additive mask tile into PSUM before the score
    matmul accumulates on top.
  - The softmax denominator comes for free from a 65th all-ones column
    appended to V (row 64 of the PV PSUM output).
  - Output projection consumes attn^T and produces out^T; the host
    transposes while unsharding.
"""

import numpy as np
import ml_dtypes

BF16 = ml_dtypes.bfloat16

B, N, C, H, D = 4, 2048, 1024, 16, 64
HPC = 8            # heads per core
GD = HPC * D       # 512 channels per head-group
P = 128
KC = C // P        # 8 contraction chunks for the projections
SPAN = 512         # query-column span processed per attention step
NSPAN = N // SPAN
NEG = -28672.0     # additive mask; exactly representable in bf16

_CACHE = {}


def _emit_once(tc, mybir, xT_d, wqkT_d, wvT_d, wpT_d, bm_d, id_d, out_d,
               phases):
    nc = tc.nc
    dt = mybir.dt
    f32, bf = dt.float32, dt.bfloat16
    Exp = mybir.ActivationFunctionType.Exp
    MUL = mybir.AluOpType.mult
    do_qkv = "qkv" in phases
    do_attn = "attn" in phases
    do_proj = "proj" in phases

    with (
        tc.tile_pool(name="weights", bufs=1) as wp,
        tc.tile_pool(name="acts", bufs=1) as ab,
        tc.tile_pool(name="small", bufs=4) as sp,
        tc.tile_pool(name="ps", bufs=1, space="PSUM") as ps,
        tc.tile_pool(name="aTp", bufs=2) as aTp,
        tc.tile_pool(name="exp", bufs=3) as exp_pool,
    ):
        # ---------------- input loads (chunked: DMA parallelism + fine deps)
        xk = [[wp.tile([P, N // 2], bf, tag=f"xk{k}_{h2}", name=f"xk{k}_{h2}")
               for h2 in range(2)] for k in range(KC)]
        wqk = [wp.tile([P, 2 * GD], bf, tag=f"wqk{k}", name=f"wqk{k}")
               for k in range(KC)]
        wv = [wp.tile([P, GD], bf, tag=f"wv{k}", name=f"wv{k}")
              for k in range(KC)]
        for k in range(KC):
            for h2 in range(2):
                nc.sync.dma_start(
                    xk[k][h2],
                    xT_d[k * P:(k + 1) * P,
                         h2 * (N // 2):(h2 + 1) * (N // 2)])
            nc.sync.dma_start(wqk[k], wqkT_d[k * P:(k + 1) * P, :])
            nc.sync.dma_start(wv[k], wvT_d[k * P:(k + 1) * P, :])
        wpk = [wp.tile([P, C], bf, tag=f"wpk{k}", name=f"wpk{k}")
               for k in range(GD // P)]
        for k in range(GD // P):
            nc.sync.dma_start(wpk[k], wpT_d[k * P:(k + 1) * P, :])
        bm = wp.tile([P, 2 * SPAN], bf, tag="bm")
        nc.sync.dma_start(bm, bm_d)
        i128 = wp.tile([P, P], bf, tag="i128")
        nc.sync.dma_start(i128, id_d)

        # q^T/k^T rows, one tile per 128-row chunk for fine-grained deps
        qkm = [ab.tile([P, N], bf, tag=f"qkm{m}", name=f"qkm{m}")
               for m in range(2 * GD // P)]
        # V per kv-block with an all-ones 65th column per head
        vab = [ab.tile([P, HPC * (D + 1)], bf, tag=f"vab{m}", name=f"vab{m}")
               for m in range(N // P)]

        # PSUM bank budget (8 banks of [128, 512]f32):
        #   qk (QKV groups)     [128, 2, 512] x1  = 2
        #   duo/pp              [128, 2, 512] x2  = 4
        #   oA, oB              [65, 512]     x1  = 2
        def qk_chunk(m):
            if not do_qkv:
                return
            for q in range(4):
                pg = ps.tile([P, SPAN], f32, tag="qk", name=f"pg{m}{q}",
                             bufs=2)
                for k in range(KC):
                    nc.tensor.matmul(
                        pg,
                        wqk[k][:, m * P:(m + 1) * P],
                        xk[k][q // 2][:, (q % 2) * SPAN:(q % 2 + 1) * SPAN],
                        start=(k == 0),
                        stop=(k == KC - 1),
                    )
                nc.vector.tensor_copy(
                    out=qkm[m][:, q * SPAN:(q + 1) * SPAN], in_=pg)

        def v_chunk(m4):
            if not do_qkv:
                return
            for sub in range(4):
                m16 = m4 * 4 + sub
                pv = ps.tile([P, SPAN], f32, tag="qk", name=f"pv{m16}",
                             bufs=2)
                nc.vector.memset(vab[m16], 1.0)
                for k in range(KC):
                    nc.tensor.matmul(
                        pv,
                        xk[k][m16 // 8][:, (m16 % 8) * P:(m16 % 8 + 1) * P],
                        wv[k],
                        start=(k == 0),
                        stop=(k == KC - 1),
                    )
                nc.vector.tensor_copy(
                    out=vab[m16].rearrange(
                        "p (h e) -> p h e", h=HPC)[:, :, :D],
                    in_=pv.rearrange("p (h e) -> p h e", h=HPC),
                )

        def attn_pair(J, hp, acT):
            if not do_attn:
                return
            nblk = 4 * (J + 1)
            qs = J * SPAN
            outs = (
                ps.tile([65, SPAN], f32, tag="oA", name="oA", bufs=1),
                ps.tile([65, SPAN], f32, tag="oB", name="oB", bufs=1),
            )
            def emit_pv(ex, j2, lo):
                for hi in (0, 1):
                    h = 2 * hp + hi
                    nc.tensor.matmul(
                        outs[hi][:, lo:],
                        vab[j2][:, h * (D + 1):(h + 1) * (D + 1)],
                        ex[:, hi, lo:],
                        start=(j2 == 0),
                        stop=(j2 == nblk - 1),
                    )

            pend = None  # software pipeline: PV one block behind scores/exp
            for j2 in range(nblk):
                duo = ps.tile([P, 2, SPAN], f32, tag="duo", bufs=2)
                dtg = j2 - 4 * J   # >=0: diagonal block index
                lo = P * dtg if dtg >= 0 else 0  # first live column
                for hi in (0, 1):
                    sl = duo[:, hi, lo:]
                    first = True
                    if dtg >= 0:
                        # triangle mask on the 128-wide diagonal
                        nc.tensor.matmul(
                            sl[:, :P], i128, bm[:, SPAN:SPAN + P],
                            start=True, stop=False,
                        )
                        first = False
                    nc.tensor.matmul(
                        sl,
                        qkm[4 + hp][64 * hi:64 * (hi + 1),
                                    j2 * P:(j2 + 1) * P],
                        qkm[hp][64 * hi:64 * (hi + 1), qs + lo:qs + SPAN],
                        start=first,
                        stop=True,
                    )
                ex = exp_pool.tile([P, 2, SPAN], bf, tag="ex")
                nc.scalar.activation(ex[:, :, lo:], duo[:, :, lo:], Exp)
                if pend is not None:
                    emit_pv(*pend)
                pend = (ex, j2, lo)
            emit_pv(*pend)
            for hi in (0, 1):
                o = outs[hi]
                rc = sp.tile([1, SPAN], f32, tag="rc")
                nc.vector.reciprocal(rc, o[64:65, :])
                bc = sp.tile([64, SPAN], f32, tag="bc")
                nc.gpsimd.partition_broadcast(bc, rc)
                nc.vector.tensor_tensor(
                    acT[64 * hi:64 * (hi + 1), hp, :], o[0:64, :], bc, MUL,
                )

        def proj_span(J, acT):
            if not do_proj:
                return
            qs = J * SPAN
            for mo in range(C // P):
                pp = ps.tile([P, SPAN], f32, tag="duo", name=f"pp{mo}",
                             bufs=2)
                for k in range(GD // P):
                    nc.tensor.matmul(
                        pp,
                        wpk[k][:, mo * P:(mo + 1) * P],
                        acT[:, k, :],
                        start=(k == 0),
                        stop=(k == GD // P - 1),
                    )
                ob = sp.tile([P, SPAN], f32, tag="ob")
                nc.vector.tensor_copy(out=ob, in_=pp)
                nc.sync.dma_start(out_d[mo * P:(mo + 1) * P, qs:qs + SPAN],
                                  ob)

        # Interleaved emission: attention (span J, pair hp) needs qkm[hp],
        # qkm[4+hp], vab[0..4J+3]; unblock hp pairs of span 0 early so ACT
        # overlaps the QKV phase.
        acTs = [aTp.tile([P, GD // P, SPAN], bf, tag="acT", name=f"acT{J}")
                for J in range(NSPAN)]
        qk_chunk(0)
        qk_chunk(4)
        v_chunk(0)
        attn_pair(0, 0, acTs[0])
        qk_chunk(1)
        qk_chunk(5)
        attn_pair(0, 1, acTs[0])
        qk_chunk(2)
        qk_chunk(6)
        attn_pair(0, 2, acTs[0])
        qk_chunk(3)
        qk_chunk(7)
        attn_pair(0, 3, acTs[0])
        v_chunk(1)
        attn_pair(1, 0, acTs[1])
        proj_span(0, acTs[0])
        for hp in range(1, 4):
            attn_pair(1, hp, acTs[1])
        v_chunk(2)
        attn_pair(2, 0, acTs[2])
        proj_span(1, acTs[1])
        for hp in range(1, 4):
            attn_pair(2, hp, acTs[2])
        v_chunk(3)
        attn_pair(3, 0, acTs[3])
        proj_span(2, acTs[2])
        for hp in range(1, 4):
            attn_pair(3, hp, acTs[3])
        proj_span(3, acTs[3])


def _emit(tc, mybir, reps=1, phases=("qkv", "attn", "proj")):
    nc = tc.nc
    dt = mybir.dt
    f32, bf = dt.float32, dt.bfloat16

    xT_d = nc.dram_tensor("xT", [C, N], bf, kind="ExternalInput").ap()
    wqkT_d = nc.dram_tensor("wqkT", [C, 2 * GD], bf, kind="ExternalInput").ap()
    wvT_d = nc.dram_tensor("wvT", [C, GD], bf, kind="ExternalInput").ap()
    wpT_d = nc.dram_tensor("wpT", [GD, C], bf, kind="ExternalInput").ap()
    bm_d = nc.dram_tensor("BM", [P, 2 * SPAN], bf, kind="ExternalInput").ap()
    id_d = nc.dram_tensor("I128", [P, P], bf, kind="ExternalInput").ap()
    out_d = nc.dram_tensor("outT", [C, N], f32, kind="ExternalOutput").ap()

    for _rep in range(reps):
        _emit_once(tc, mybir, xT_d, wqkT_d, wvT_d, wpT_d, bm_d, id_d, out_d,
                   phases)


def _get_module(reps=1, phases=("qkv", "attn", "proj")):
    key = (reps, tuple(phases))
    if key not in _CACHE:
        import concourse.tile as tile
        from concourse import bacc, mybir

        nc = bacc.Bacc("TRN2", target_bir_lowering=False, debug=False,
                       num_devices=8)
        with tile.TileContext(nc) as tc:
            _emit(tc, mybir, reps=reps, phases=phases)
        nc.compile()
        _CACHE[key] = nc
    return _CACHE[key]


def _host_inputs(x, w_qkv, w_proj):
    scale = D ** -0.5
    bmask = np.full((P, 2 * SPAN), NEG, np.float32)
    for p in range(P):
        bmask[p, p + SPAN:] = 0.0
    bmask = bmask.astype(BF16)
    ident = np.eye(P, dtype=BF16)
    in_maps = []
    for core in range(8):
        b, g = core // 2, core % 2
        rows = slice(g * GD, (g + 1) * GD)
        wq = w_qkv[0 * C:1 * C][rows] * scale
        wk = w_qkv[1 * C:2 * C][rows]
        wv = w_qkv[2 * C:3 * C][rows]
        in_maps.append({
            "xT": np.ascontiguousarray(x[b].T).astype(BF16),
            "wqkT": np.ascontiguousarray(
                np.concatenate([wq, wk], axis=0).T).astype(BF16),
            "wvT": np.ascontiguousarray(wv.T).astype(BF16),
            "wpT": np.ascontiguousarray(w_proj[:, rows].T).astype(BF16),
            "BM": bmask,
            "I128": ident,
        })
    return in_maps


def kernel(x, w_qkv, w_proj, b_proj, _trace=False):
    from concourse.bass_utils import run_bass_kernel_spmd

    nc = _get_module()
    in_maps = _host_inputs(np.asarray(x, np.float32),
                           np.asarray(w_qkv, np.float32),
                           np.asarray(w_proj, np.float32))
    res = run_bass_kernel_spmd(nc, in_maps, core_ids=list(range(8)),
                               trace=_trace)
    outs = [r["outT"] for r in res.results]
    out = np.empty((B, N, C), np.float32)
    bp = np.asarray(b_proj, np.float32)[None, :]
    for b in range(B):
        out[b] = outs[2 * b].T + outs[2 * b + 1].T + bp
    if _trace:
        kernel._last_results = res
    return out
